# revision 1
# baseline (speedup 1.0000x reference)
"""Trainium2 Bass kernel for nn_AttModel_self_syb (dense transformer, 6 blocks).

Sharding: data-parallel over batch. 16 batches -> 8 NeuronCores x 2 batches
(512 tokens per core), full weights on every core, zero collectives.
The 401k x 300 embedding table is "gather-sharded" on the host: each core only
receives the (512, 300) rows its tokens reference (pure input sharding).

On-device dataflow is entirely FEATURE-MAJOR ([feature_partition, token_free]),
which removes every transpose:
  - y = x @ W           -> matmul(lhsT=W[k,m], rhs=xT[k,tok]) = yT
  - v (token-major)     -> matmul(lhsT=xT[k,tok_chunk], rhs=wv[k,n])
  - scores sT=[k_tok,q] -> matmul(lhsT=kT_head[dh,k_chunk], rhs=qT_head[dh,q])
  - softmax             -> exp(s/sqrt(dh)) * mask (no max-subtraction; scores
                           are O(1) here), normalizer from an extra ones-column
                           carried in the v tile, applied via reciprocal +
                           gpsimd partition_broadcast
  - LayerNorm           -> per-token stats across the partition axis via
                           ones-vector matmuls on TensorE (f32r), rstd via
                           exp(-0.5*ln(var+eps)) (stays in one ACT table set)
Matmul operands are bf16 (fp32 PSUM accumulation); the residual stream, all
statistics and softmax normalization stay fp32.
"""

import os
import contextlib

import numpy as np
import ml_dtypes

import concourse.bass as bass
from concourse import bacc
import concourse.mybir as mybir
import concourse.tile as tile
from concourse.bass_utils import run_bass_kernel_spmd

F32 = mybir.dt.float32
F32R = mybir.dt.float32r
BF16 = mybir.dt.bfloat16
AF = mybir.ActivationFunctionType
ALU = mybir.AluOpType

# model dims (hardcoded per problem spec)
B, T, D, H, NB = 16, 256, 1024, 16, 6
V, GD, MLP_H, FF_H = 401000, 300, 2048, 4096
DH = D // H                    # 64
NCORES = 8
BPC = B // NCORES              # 2 batches per core
N = BPC * T                    # 512 tokens per core
SCALE = 1.0 / float(np.sqrt(DH))
EPS = 1e-8

CDT = BF16                     # matmul-operand dtype
NPCDT = ml_dtypes.bfloat16

P = 128
DT_TILES = D // P              # 8
FF_TILES = FF_H // P           # 32
HT = T // P                    # 2 key chunks per batch
NT = N // P                    # 4 token tiles per core
VH = DH + 1                    # per-head v columns incl. ones column
VCOLS = H * VH                 # 1040

N_BLOCKS = int(os.environ.get("BASS_KERNEL_NBLOCKS", NB))


def build_graph(use_bv: bool, ln_affine: bool = True):
    nc = bacc.Bacc()
    g = {}
    g["eT"] = nc.declare_dram_parameter("eT", [GD, N], CDT, isOutput=False)
    g["posT"] = nc.declare_dram_parameter("posT", [D, N], F32, isOutput=False)
    g["maskT"] = nc.declare_dram_parameter("maskT", [BPC, T, T], CDT, isOutput=False)
    g["qmask"] = nc.declare_dram_parameter("qmask", [BPC, T], F32, isOutput=False)

    g["mlp_w1"] = nc.declare_dram_parameter("mlp_w1", [GD, MLP_H], CDT, isOutput=False)
    g["mlp_b1"] = nc.declare_dram_parameter("mlp_b1", [MLP_H], F32, isOutput=False)
    g["mlp_w2"] = nc.declare_dram_parameter("mlp_w2", [MLP_H, D], CDT, isOutput=False)
    g["mlp_b2"] = nc.declare_dram_parameter("mlp_b2", [D], F32, isOutput=False)

    for nm, shp in (("wq", [NB, D, D]), ("wk", [NB, D, D]), ("wv", [NB, D, D]),
                    ("ff_w1", [NB, D, FF_H]), ("ff_w2", [NB, FF_H, D])):
        g[nm] = nc.declare_dram_parameter(nm, shp, CDT, isOutput=False)
    for nm, shp in (("bq", [NB, D]), ("bk", [NB, D]), ("bv", [NB, D]),
                    ("ff_b1", [NB, FF_H]), ("ff_b2", [NB, D]),
                    ("ln1_g", [NB, D]), ("ln1_b", [NB, D]),
                    ("ln2_g", [NB, D]), ("ln2_b", [NB, D])):
        g[nm] = nc.declare_dram_parameter(nm, shp, F32, isOutput=False)

    g["ones"] = nc.declare_dram_parameter("ones", [P, 1], F32R, isOutput=False)
    g["out"] = nc.declare_dram_parameter("out", [D, N], F32, isOutput=True)

    with tile.TileContext(nc) as tc:
        _body(nc, tc, g, use_bv, ln_affine)
    nc.finalize()
    return nc


def _body(nc, tc, g, use_bv, ln_affine):
    ctx = contextlib.ExitStack()
    with ctx:
        # ---- SBUF pools (per-partition bytes in comments) ----
        wbig = ctx.enter_context(tc.tile_pool(name="wbig", bufs=11))   # 4KB*12 = 48KB
        h1p = ctx.enter_context(tc.tile_pool(name="h1p", bufs=1))      # 32KB
        xbp = ctx.enter_context(tc.tile_pool(name="xbp", bufs=1))      # 1KB*8 = 8KB
        xfp = ctx.enter_context(tc.tile_pool(name="xfp", bufs=1))      # 2KB*8 = 16KB
        qkp = ctx.enter_context(tc.tile_pool(name="qkp", bufs=1))      # 1KB*16 = 16KB
        vp = ctx.enter_context(tc.tile_pool(name="vp", bufs=1))        # ~2KB*4 = 8.2KB
        esp = ctx.enter_context(tc.tile_pool(name="esp", bufs=8))      # 0.5KB*8 = 4KB
        rp = ctx.enter_context(tc.tile_pool(name="rp", bufs=1))        # 2KB*8 = 16KB
        op = ctx.enter_context(tc.tile_pool(name="op", bufs=1))        # 2KB*8 = 16KB
        sqp = ctx.enter_context(tc.tile_pool(name="sqp", bufs=4))      # 2KB*4 = 8KB
        bcp = ctx.enter_context(tc.tile_pool(name="bcp", bufs=3))      # 2KB*3 = 6KB
        bhp = ctx.enter_context(tc.tile_pool(name="bhp", bufs=5))      # 1KB*5 = 5KB
        rowp = ctx.enter_context(tc.tile_pool(name="rowp", bufs=1))    # tiny
        cstp = ctx.enter_context(tc.tile_pool(name="cstp", bufs=2))    # tiny
        onep = ctx.enter_context(tc.tile_pool(name="onep", bufs=1))    # consts/masks

        # ---- PSUM: one bank per [128,512] fp32 tile ----
        psp = ctx.enter_context(tc.tile_pool(name="psp", bufs=8, space="PSUM"))
        rsp = psp

        def ps_tile(name):
            return psp.tile([P, N], F32, name=name, tag="mm")

        ones_col = onep.tile([P, 1], F32R, name="ones_col", tag="ones_col")
        nc.sync.dma_start(out=ones_col, in_=g["ones"][:, :])

        qm_rows = []
        for b in range(BPC):
            qm_b = onep.tile([1, T], F32, name=f"qm_{b}", tag=f"qm_{b}")
            nc.sync.dma_start(out=qm_b, in_=g["qmask"][b:b + 1, :])
            qm_rows.append(qm_b)

        mtiles = {}
        for b in range(BPC):
            for kc in range(HT):
                mt = onep.tile([P, T], CDT, name=f"mask_{b}_{kc}", tag=f"mask_{b}_{kc}")
                nc.sync.dma_start(out=mt, in_=g["maskT"][b, kc * P:(kc + 1) * P, :])
                mtiles[(b, kc)] = mt

        def bias_bundle(vec_ap, ncols, name):
            """[ncols*128] DRAM vector -> [128, ncols] sbuf; column m = slice m."""
            tl = cstp.tile([P, ncols], F32, name=name, tag="bias_bundle", bufs=6)
            nc.sync.dma_start(out=tl, in_=vec_ap.rearrange("(m p) -> p m", p=P))
            return tl

        # =============== embedding MLP ===============
        GK = [(0, 128), (128, 128), (256, GD - 256)]
        e_tiles = []
        for i, (k0, kn) in enumerate(GK):
            et = wbig.tile([P, 2048], CDT, name=f"et_{i}", tag="wbig")
            nc.sync.dma_start(out=et[:kn, :N], in_=g["eT"][k0:k0 + kn, :])
            e_tiles.append((et, kn))
        w1t = []
        for i, (k0, kn) in enumerate(GK):
            w = wbig.tile([P, 2048], CDT, name=f"mw1_{i}", tag="wbig")
            nc.sync.dma_start(out=w[:kn, :], in_=g["mlp_w1"][k0:k0 + kn, :])
            w1t.append((w, kn))
        mb1 = bias_bundle(g["mlp_b1"][:], MLP_H // P, "mb1")

        h0 = h1p.tile([P, FF_TILES * N], CDT, name="h0", tag="h1")
        for m in range(MLP_H // P):
            ps = ps_tile("mlp1_ps")
            for i, (k0, kn) in enumerate(GK):
                nc.tensor.matmul(ps, w1t[i][0][:kn, m * P:(m + 1) * P],
                                 e_tiles[i][0][:kn, :N],
                                 start=(i == 0), stop=(i == len(GK) - 1))
            nc.scalar.activation(h0[:, m * N:(m + 1) * N], ps, AF.Relu,
                                 bias=mb1[:, m:m + 1])

        mb2 = bias_bundle(g["mlp_b2"][:], DT_TILES, "mb2")
        x_bf = [xbp.tile([P, N], CDT, name=f"x0b_{m}", tag=f"x_{m}") for m in range(DT_TILES)]
        x_f32 = [xfp.tile([P, N], F32, name=f"x0f_{m}", tag=f"xf_{m}") for m in range(DT_TILES)]
        MK = MLP_H // P  # 16 k-tiles, in 2 groups of 8
        pss = {m: ps_tile(f"mlp2_ps_{m}") for m in range(DT_TILES)}
        for kg in range(2):
            w2t = []
            for j in range(8):
                k = kg * 8 + j
                w = wbig.tile([P, 2048], CDT, name=f"mw2_{k}", tag="wbig")
                nc.sync.dma_start(out=w[:, :D], in_=g["mlp_w2"][k * P:(k + 1) * P, :])
                w2t.append(w)
            for j in range(8):
                k = kg * 8 + j
                for m in range(DT_TILES):
                    nc.tensor.matmul(pss[m], w2t[j][:, m * P:(m + 1) * P],
                                     h0[:, k * N:(k + 1) * N],
                                     start=(k == 0), stop=(k == MK - 1))
        for m in range(DT_TILES):
            pos_m = bcp.tile([P, N], F32, name=f"pos_{m}", tag="bc")
            nc.sync.dma_start(out=pos_m, in_=g["posT"][m * P:(m + 1) * P, :])
            nc.vector.scalar_tensor_tensor(x_f32[m], pss[m], mb2[:, m:m + 1], pos_m,
                                           op0=ALU.add, op1=ALU.add)
            nc.vector.tensor_copy(x_bf[m], x_f32[m])

        r_cur = x_f32  # fp32 residual stream

        # =============== transformer blocks ===============
        for blk in range(N_BLOCKS):
            bq_b = bias_bundle(g["bq"][blk, :], DT_TILES, f"bq_{blk}")
            bk_b = bias_bundle(g["bk"][blk, :], DT_TILES, f"bk_{blk}")

            # ---- q/k projections, feature-major ----
            qT = [qkp.tile([P, N], CDT, name=f"q{blk}_{m}", tag=f"q_{m}") for m in range(DT_TILES)]
            kTt = [qkp.tile([P, N], CDT, name=f"k{blk}_{m}", tag=f"k_{m}") for m in range(DT_TILES)]
            for wname, bb, dst in (("wq", bq_b, qT), ("wk", bk_b, kTt)):
                wt = []
                for k in range(DT_TILES):
                    w = wbig.tile([P, 2048], CDT, name=f"{wname}{blk}_{k}", tag="wbig")
                    nc.sync.dma_start(out=w[:, :D], in_=g[wname][blk, k * P:(k + 1) * P, :])
                    wt.append(w)
                for m in range(DT_TILES):
                    ps = ps_tile(f"{wname}_ps")
                    for k in range(DT_TILES):
                        nc.tensor.matmul(ps, wt[k][:, m * P:(m + 1) * P], x_bf[k],
                                         start=(k == 0), stop=(k == DT_TILES - 1))
                    nc.scalar.activation(dst[m], ps, AF.Relu, bias=bb[:, m:m + 1])

            # ---- v projection, token-major, per-head layout with ones cols ----
            wvt = []
            for k in range(DT_TILES):
                w = wbig.tile([P, 2048], CDT, name=f"wv{blk}_{k}", tag="wbig")
                nc.sync.dma_start(out=w[:, :D], in_=g["wv"][blk, k * P:(k + 1) * P, :])
                wvt.append(w)
            if use_bv:
                bv_row = rowp.tile([1, D], F32, name=f"bvr_{blk}", tag="row_bv", bufs=1)
                nc.sync.dma_start(out=bv_row, in_=g["bv"][blk:blk + 1, :])
                bv_bc = bcp.tile([P, D], F32, name=f"bvb_{blk}", tag="bc_bv", bufs=2)
                nc.gpsimd.partition_broadcast(bv_bc, bv_row)
            vt = [vp.tile([P, VCOLS], CDT, name=f"v{blk}_{tt}", tag=f"v_{tt}") for tt in range(NT)]
            for tt in range(NT):
                ones_ap = vt[tt].rearrange("p (h c) -> p h c", h=H)[:, :, DH:VH]
                nc.vector.memset(ones_ap, 1.0)
                for half in range(2):
                    ps = ps_tile("v_ps")
                    c0 = half * (D // 2)
                    for k in range(DT_TILES):
                        nc.tensor.matmul(ps, x_bf[k][:, tt * P:(tt + 1) * P],
                                         wvt[k][:, c0:c0 + D // 2],
                                         start=(k == 0), stop=(k == DT_TILES - 1))
                    dst = vt[tt].rearrange("p (h c) -> p h c", h=H)[
                        :, half * (H // 2):(half + 1) * (H // 2), 0:DH]
                    src = ps[:, :D // 2]
                    if use_bv:
                        tmp = sqp.tile([P, D // 2], F32, name="v_tmp", tag="sq")
                        nc.vector.tensor_add(tmp, src, bv_bc[:, c0:c0 + D // 2])
                        src = tmp
                    nc.scalar.activation(
                        dst, src.rearrange("p (h c) -> p h c", c=DH), AF.Relu)

            # ---- attention ----
            o_acc = [op.tile([P, N], F32, name=f"o{blk}_{m}", tag=f"o_{m}") for m in range(DT_TILES)]

            def emit_scores(b, h):
                ft, fo = h // 2, (h % 2) * DH
                es = []
                for kc in range(HT):
                    ps = psp.tile([P, N], F32, name="s_ps", tag="mm")
                    nc.tensor.matmul(
                        ps[:, :T],
                        kTt[ft][fo:fo + DH, b * T + kc * P: b * T + (kc + 1) * P],
                        qT[ft][fo:fo + DH, b * T:(b + 1) * T],
                        start=True, stop=True)
                    ex = esp.tile([P, T], CDT, name="expS", tag="es")
                    nc.scalar.activation(ex, ps[:, :T], AF.Exp, scale=SCALE)
                    exm = esp.tile([P, T], CDT, name="expSm", tag="es")
                    nc.vector.tensor_mul(exm, ex, mtiles[(b, kc)])
                    es.append(exm)
                return es

            def emit_out(b, h, es):
                ft, fo = h // 2, (h % 2) * DH
                ops_t = psp.tile([P, N], F32, name="o_head_ps", tag="mm")
                for kc in range(HT):
                    nc.tensor.matmul(ops_t[:VH, :T],
                                     vt[(b * T) // P + kc][:, h * VH:(h + 1) * VH],
                                     es[kc],
                                     start=(kc == 0), stop=(kc == HT - 1))
                # normalizer: qmask / (denom + tiny)
                den = rowp.tile([1, T], F32, name="den", tag="row_t", bufs=4)
                nc.vector.tensor_scalar_add(den, ops_t[DH:VH, :T], 1e-30)
                nc.vector.reciprocal_approx_fast(den, den)
                mrow = rowp.tile([1, T], F32, name="mrow", tag="row_m", bufs=4)
                nc.vector.tensor_mul(mrow, den, qm_rows[b])
                brow = bhp.tile([DH, T], F32, name="brow", tag="bc_h")
                nc.gpsimd.partition_broadcast(brow, mrow)
                nc.vector.tensor_mul(
                    o_acc[ft][fo:fo + DH, b * T:(b + 1) * T], ops_t[0:DH, :T], brow)

            LOOKAHEAD = 3
            heads = [(b, h) for b in range(BPC) for h in range(H)]
            queue = []
            for bh in heads:
                queue.append((bh, emit_scores(*bh)))
                if len(queue) > LOOKAHEAD:
                    (pb, ph), pes = queue.pop(0)
                    emit_out(pb, ph, pes)
            for (pb, ph), pes in queue:
                emit_out(pb, ph, pes)

            # ---- residual 1 + LN1 ----
            r_new = [rp.tile([P, N], F32R, name=f"r1_{blk}_{m}", tag=f"r_{m}") for m in range(DT_TILES)]
            for m in range(DT_TILES):
                nc.vector.tensor_add(r_new[m], o_acc[m], r_cur[m])
            x_bf, x_f32 = _layernorm(nc, g, blk, "ln1", r_new, ones_col,
                                     xbp, xfp, sqp, bcp, rowp, cstp, rsp, psp, None,
                                     ln_affine)
            r_cur = x_f32

            # ---- FFN up (2 column passes) ----
            fb1 = bias_bundle(g["ff_b1"][blk, :], FF_TILES, f"fb1_{blk}")
            h1 = h1p.tile([P, FF_TILES * N], CDT, name=f"h1_{blk}", tag="h1")
            for ph in range(2):
                w1t = []
                for k in range(DT_TILES):
                    w = wbig.tile([P, 2048], CDT, name=f"fw1_{blk}_{ph}_{k}", tag="wbig")
                    nc.sync.dma_start(
                        out=w, in_=g["ff_w1"][blk, k * P:(k + 1) * P,
                                              ph * 2048:(ph + 1) * 2048])
                    w1t.append(w)
                for mm in range(16):
                    m = ph * 16 + mm
                    ps = ps_tile("ff1_ps")
                    for k in range(DT_TILES):
                        nc.tensor.matmul(ps, w1t[k][:, mm * P:(mm + 1) * P], x_bf[k],
                                         start=(k == 0), stop=(k == DT_TILES - 1))
                    nc.scalar.activation(h1[:, m * N:(m + 1) * N], ps, AF.Relu,
                                         bias=fb1[:, m:m + 1])

            # ---- FFN down (2 output halves, streaming k-groups) ----
            fb2 = bias_bundle(g["ff_b2"][blk, :], DT_TILES, f"fb2_{blk}")
            r_new = [rp.tile([P, N], F32R, name=f"r2_{blk}_{m}", tag=f"r_{m}") for m in range(DT_TILES)]
            pss = {m: ps_tile(f"ff2_ps_{m}") for m in range(DT_TILES)}
            for kg in range(4):
                w2t = []
                for j in range(8):
                    k = kg * 8 + j
                    w = wbig.tile([P, 2048], CDT, name=f"fw2_{blk}_{k}", tag="wbig")
                    nc.sync.dma_start(out=w[:, :D],
                                      in_=g["ff_w2"][blk, k * P:(k + 1) * P, :])
                    w2t.append(w)
                for j in range(8):
                    k = kg * 8 + j
                    for m in range(DT_TILES):
                        nc.tensor.matmul(pss[m], w2t[j][:, m * P:(m + 1) * P],
                                         h1[:, k * N:(k + 1) * N],
                                         start=(k == 0), stop=(k == FF_TILES - 1))
            for m in range(DT_TILES):
                # r2 = (ff2 + b2) + x_postLN1
                nc.vector.scalar_tensor_tensor(r_new[m], pss[m], fb2[:, m:m + 1],
                                               x_f32[m], op0=ALU.add, op1=ALU.add)
            last = blk == N_BLOCKS - 1
            x_bf, x_f32 = _layernorm(nc, g, blk, "ln2", r_new, ones_col,
                                     xbp, xfp, sqp, bcp, rowp, cstp, rsp, psp,
                                     g["out"] if last else None, ln_affine)
            r_cur = x_f32


def _layernorm(nc, g, blk, which, r_tiles, ones_col,
               xbp, xfp, sqp, bcp, rowp, cstp, rsp, psp, out_dram, affine):
    nt = len(r_tiles)
    if affine:
        gb = cstp.tile([P, nt], F32, name=f"{which}g_{blk}", tag="bias_bundle", bufs=6)
        nc.sync.dma_start(out=gb, in_=g[f"{which}_g"][blk, :].rearrange("(m p) -> p m", p=P))
        bb = cstp.tile([P, nt], F32, name=f"{which}b_{blk}", tag="bias_bundle", bufs=6)
        nc.sync.dma_start(out=bb, in_=g[f"{which}_b"][blk, :].rearrange("(m p) -> p m", p=P))

    sums = rsp.tile([P, N], F32, name=f"{which}_sum_{blk}", tag="mm")[0:1, :]
    sumsq = rsp.tile([P, N], F32, name=f"{which}_sumsq_{blk}", tag="mm")[0:1, :]
    oc = ones_col
    for m in range(nt):
        nc.tensor.matmul(sums, oc, r_tiles[m],
                         start=(m == 0), stop=(m == nt - 1))
    for m in range(nt):
        s = sqp.tile([P, N], F32R, name=f"{which}_sq", tag="sq")
        nc.scalar.square(s, r_tiles[m])
        nc.tensor.matmul(sumsq, oc, s,
                         start=(m == 0), stop=(m == nt - 1))

    mean = rowp.tile([1, N], F32, name=f"{which}_mean", tag="row_a")
    nc.scalar.mul(mean, sums, 1.0 / D)
    # var = sumsq/D - mean^2, fused: t = -mean*mean ; var = (sumsq*(1/D)) + t
    t = rowp.tile([1, N], F32, name=f"{which}_t", tag="row_b")
    nc.vector.scalar_tensor_tensor(t, mean, -1.0, mean, op0=ALU.mult, op1=ALU.mult)
    var = rowp.tile([1, N], F32, name=f"{which}_var", tag="row_c")
    nc.vector.scalar_tensor_tensor(var, sumsq, 1.0 / D, t, op0=ALU.mult, op1=ALU.add)
    # rstd = exp(-0.5*ln(var+eps)) -- Ln/Exp share an ACT table set (no swaps)
    eps_c = rowp.tile([1, 1], F32, name=f"{which}_eps", tag="row_eps")
    nc.vector.memset(eps_c, EPS)
    lnv = rowp.tile([1, N], F32, name=f"{which}_lnv", tag="row_d")
    nc.scalar.activation(lnv, var, AF.Ln, bias=eps_c)
    rstd = rowp.tile([1, N], F32, name=f"{which}_rstd", tag="row_e")
    nc.scalar.activation(rstd, lnv, AF.Exp, scale=-0.5)
    b_mean = bcp.tile([P, N], F32, name=f"{which}_bmean", tag="bc")
    nc.gpsimd.partition_broadcast(b_mean, mean)   # overlaps the var/rstd chain
    b_rstd = bcp.tile([P, N], F32, name=f"{which}_brstd", tag="bc")
    nc.gpsimd.partition_broadcast(b_rstd, rstd)

    xb_out = []
    for m in range(nt):
        t1 = sqp.tile([P, N], F32, name=f"{which}_t1", tag="sq")
        nc.vector.tensor_sub(t1, r_tiles[m], b_mean)
        if out_dram is not None:
            xo = sqp.tile([P, N], F32, name=f"{which}_xo", tag="sq")
            nc.vector.tensor_mul(xo, t1, b_rstd)
            if affine:
                nc.vector.tensor_scalar(out=xo, in0=xo, scalar1=gb[:, m:m + 1],
                                        scalar2=bb[:, m:m + 1], op0=ALU.mult, op1=ALU.add)
            nc.sync.dma_start(out=out_dram[m * P:(m + 1) * P, :], in_=xo)
            xb_out.append(None)
        else:
            xb = xbp.tile([P, N], CDT, name=f"{which}_xb_{m}", tag=f"x_{m}")
            if affine:
                xf = sqp.tile([P, N], F32, name=f"{which}_xf", tag="sq")
                nc.vector.tensor_mul(xf, t1, b_rstd)
                nc.vector.tensor_scalar(out=xb, in0=xf, scalar1=gb[:, m:m + 1],
                                        scalar2=bb[:, m:m + 1], op0=ALU.mult, op1=ALU.add)
            else:
                nc.vector.tensor_mul(xb, t1, b_rstd)
            xb_out.append(xb)
    return xb_out, xb_out


# ---------------------------------------------------------------------------
# host side
# ---------------------------------------------------------------------------

def _prepare_inputs(inputs):
    ipt = np.asarray(inputs["syb_ipt"]).astype(np.int64)
    emb = np.asarray(inputs["emb_table"], dtype=np.float32)
    smask = np.asarray(inputs["syb_mask"]).astype(np.int32)
    graph = np.asarray(inputs["syb_graph"]).astype(np.int32)

    gathered = emb[ipt]                                   # (B, T, GD)
    km = smask > 0
    M = (graph > 0) & km[:, None, :]                      # (B, Tq, Tk)
    MT = np.transpose(M, (0, 2, 1)).astype(NPCDT)         # (B, Tk, Tq)
    qs = smask.astype(np.float32)

    posT = np.asarray(inputs["pos_table"], np.float32).T  # (D, T)
    posT2 = np.ascontiguousarray(np.tile(posT, (1, BPC)))

    def cvt(x):
        return np.ascontiguousarray(np.asarray(x, np.float32).astype(NPCDT))

    def f32(x):
        return np.ascontiguousarray(np.asarray(x, np.float32))

    common = {
        "posT": posT2,
        "ones": np.ones((P, 1), np.float32),
        "mlp_w1": cvt(inputs["mlp_w1"]), "mlp_b1": f32(inputs["mlp_b1"]),
        "mlp_w2": cvt(inputs["mlp_w2"]), "mlp_b2": f32(inputs["mlp_b2"]),
        "wq": cvt(inputs["wq"]), "wk": cvt(inputs["wk"]), "wv": cvt(inputs["wv"]),
        "bq": f32(inputs["bq"]), "bk": f32(inputs["bk"]), "bv": f32(inputs["bv"]),
        "ff_w1": cvt(inputs["ff_w1"]), "ff_b1": f32(inputs["ff_b1"]),
        "ff_w2": cvt(inputs["ff_w2"]), "ff_b2": f32(inputs["ff_b2"]),
        "ln1_g": f32(inputs["ln1_g"]), "ln1_b": f32(inputs["ln1_b"]),
        "ln2_g": f32(inputs["ln2_g"]), "ln2_b": f32(inputs["ln2_b"]),
    }
    use_bv = bool(np.any(np.asarray(inputs["bv"]) != 0))
    ln_affine = bool(
        np.any(np.asarray(inputs["ln1_g"]) != 1) or np.any(np.asarray(inputs["ln1_b"]) != 0)
        or np.any(np.asarray(inputs["ln2_g"]) != 1) or np.any(np.asarray(inputs["ln2_b"]) != 0))

    in_maps = []
    for c in range(NCORES):
        b0 = c * BPC
        eT_c = np.ascontiguousarray(gathered[b0:b0 + BPC].reshape(N, GD).T).astype(NPCDT)
        in_maps.append({
            "eT": eT_c,
            "maskT": np.ascontiguousarray(MT[b0:b0 + BPC]),
            "qmask": np.ascontiguousarray(qs[b0:b0 + BPC]),
            **common,
        })
    return in_maps, use_bv, ln_affine


def _ensure_ntff_hook():
    """The agent image's antenv package lacks axon_hooks; synthesize it so
    run_bass_kernel_spmd(trace=True) can NTFF-profile through libaxon."""
    try:
        from antenv.axon_hooks import get_axon_ntff_profile_hook  # noqa: F401
        return
    except ImportError:
        pass
    try:
        import sys
        import types
        import antenv
        from trn_agent_boot.trn_boot import _ntff_profile_via_ctypes
        hook = _ntff_profile_via_ctypes("/opt/axon/libaxon_pjrt.so")
        mod = types.ModuleType("antenv.axon_hooks")
        mod._hook = hook
        mod.get_axon_ntff_profile_hook = lambda: mod._hook
        def _set(h):
            mod._hook = h
        mod.set_axon_ntff_profile_hook = _set
        sys.modules["antenv.axon_hooks"] = mod
        antenv.axon_hooks = mod
    except Exception as e:  # profiling is best-effort
        print(f"ntff hook injection failed: {e}")


def run(inputs, trace=False, tmpdir=None):
    in_maps, use_bv, ln_affine = _prepare_inputs(inputs)
    nc = build_graph(use_bv, ln_affine)
    if trace:
        _ensure_ntff_hook()
    res = run_bass_kernel_spmd(nc, in_maps, core_ids=list(range(NCORES)),
                               trace=trace, tmpdir=tmpdir)
    out = np.empty((B, T, D), np.float32)
    for c in range(NCORES):
        xT = np.asarray(res.results[c]["out"])            # (D, N)
        out[c * BPC:(c + 1) * BPC] = xT.T.reshape(BPC, T, D)
    return out, res


def kernel(**inputs):
    out, _ = run(inputs, trace=False)
    return out



# revision 10
# speedup vs baseline: 1.1854x; 1.1854x over previous
"""Trainium2 Bass kernel for nn_AttModel_self_syb (dense transformer, 6 blocks).

Sharding: data-parallel over batch. 16 batches -> 8 NeuronCores x 2 batches
(512 tokens per core), full weights on every core, zero collectives.
The 401k x 300 embedding table is "gather-sharded" on the host: each core only
receives the (512, 300) rows its tokens reference (pure input sharding).

Feature-major on-device dataflow ([feature_partition, token_free]); v and
attention weights token-major. Perf structure (v2):
  - single ACT table set (natural_log_exp_and_others has exp/ln/relu/square/
    copy) -> no ACT_TABLE_LOADs in steady state
  - attention mask applied by accumulating NEG*(1-mask) into the score PSUM
    via an identity-weight matmul; exp() then yields exact zeros -> no DVE
    mask multiply
  - per-head softmax denominators (from a ones-column in v) collected into a
    [16, 512] tile; one reciprocal; broadcast to the o feature layout with
    tiny selection-matrix matmuls on the (otherwise idle) PE
  - LayerNorm: mean/rstd rows broadcast via ones-row PE matmuls (no gpsimd),
    rstd = exp(-0.5*ln(var+eps)) with no table swaps, residual kept
    mean-shifted in place (LN is invariant to per-token shifts)
  - FFN-down final k-group runs m-major so PSUM drains/squares/stat matmuls
    overlap the tail; weight DMAs use 2KB-per-partition tiles in groups of 8
    with a deep (24-buf) pool so the sync DMA queue prefetches ~2 groups ahead
Matmul operands are bf16 (fp32 PSUM accumulation); residual and statistics
stay fp32.
"""

import os
import contextlib

import numpy as np
import ml_dtypes

import concourse.bass as bass
from concourse import bacc
import concourse.mybir as mybir
import concourse.tile as tile
from concourse.bass_utils import run_bass_kernel_spmd

F32 = mybir.dt.float32
F32R = mybir.dt.float32r
BF16 = mybir.dt.bfloat16
AF = mybir.ActivationFunctionType
ALU = mybir.AluOpType

# model dims (hardcoded per problem spec)
B, T, D, H, NB = 16, 256, 1024, 16, 6
V, GD, MLP_H, FF_H = 401000, 300, 2048, 4096
DH = D // H                    # 64
NCORES = 8
BPC = B // NCORES              # 2 batches per core
N = BPC * T                    # 512 tokens per core
SCALE = 1.0 / float(np.sqrt(DH))
EPS = 1e-8
NEG = float(-(2**32) + 1)

CDT = BF16
NPCDT = ml_dtypes.bfloat16

P = 128
DT_TILES = D // P              # 8
FF_TILES = FF_H // P           # 32
HT = T // P                    # 2 key chunks per batch
VH = DH + 1                    # per-head v columns incl. ones column
VCOLS = H * VH                 # 1040

N_BLOCKS = int(os.environ.get("BASS_KERNEL_NBLOCKS", NB))


def _prime_act_tables(arch):
    """Collapse the activation-table choice to natural_log_exp_and_others,
    which contains every function this kernel uses (exp, ln, relu, square,
    copy, identity).  get_activation_tables() is functools.cached and the
    table-load pass reads the cached dict, so removing those functions from
    all other sets makes the pass emit a single table load."""
    try:
        from concourse.hw_specs import get_activation_tables
        tabs = get_activation_tables(arch)
        keep = "natural_log_exp_and_others"
        if keep not in tabs:
            return
        kept = set(tabs[keep])
        for name, s in tabs.items():
            if name != keep:
                s -= kept
    except Exception as e:  # pragma: no cover - best effort
        print(f"act table priming failed: {e}")


def build_graph(use_bv: bool, ln_affine: bool, use_bias: bool):
    nc = bacc.Bacc()
    _prime_act_tables(nc.m.arch)
    g = {}
    g["eT"] = nc.declare_dram_parameter("eT", [GD, N], CDT, isOutput=False)
    g["posT"] = nc.declare_dram_parameter("posT", [D, N], F32, isOutput=False)
    g["maskneg"] = nc.declare_dram_parameter("maskneg", [HT, P, N], CDT, isOutput=False)
    g["qm16"] = nc.declare_dram_parameter("qm16", [H, N], F32, isOutput=False)
    g["ssel"] = nc.declare_dram_parameter("ssel", [H, D], F32R, isOutput=False)
    g["ident"] = nc.declare_dram_parameter("ident", [P, P], CDT, isOutput=False)
    g["ones_col"] = nc.declare_dram_parameter("ones_col", [P, 1], F32R, isOutput=False)
    g["ones_row"] = nc.declare_dram_parameter("ones_row", [1, P], F32R, isOutput=False)

    g["mlp_w1"] = nc.declare_dram_parameter("mlp_w1", [GD, MLP_H], CDT, isOutput=False)
    g["mlp_b1"] = nc.declare_dram_parameter("mlp_b1", [MLP_H], F32, isOutput=False)
    g["mlp_w2"] = nc.declare_dram_parameter("mlp_w2", [MLP_H, D], CDT, isOutput=False)
    g["mlp_b2"] = nc.declare_dram_parameter("mlp_b2", [D], F32, isOutput=False)

    for nm, shp in (("wq", [NB, D, D]), ("wk", [NB, D, D]), ("wv", [NB, D, D]),
                    ("ff_w1", [NB, D, FF_H]), ("ff_w2", [NB, FF_H, D])):
        g[nm] = nc.declare_dram_parameter(nm, shp, CDT, isOutput=False)
    for nm, shp in (("bq", [NB, D]), ("bk", [NB, D]), ("bv", [NB, D]),
                    ("ff_b1", [NB, FF_H]), ("ff_b2", [NB, D]),
                    ("ln1_g", [NB, D]), ("ln1_b", [NB, D]),
                    ("ln2_g", [NB, D]), ("ln2_b", [NB, D])):
        g[nm] = nc.declare_dram_parameter(nm, shp, F32, isOutput=False)

    g["out"] = nc.declare_dram_parameter("out", [D, N], F32, isOutput=True)

    with tile.TileContext(nc) as tc:
        _body(nc, tc, g, use_bv, ln_affine, use_bias)
    nc.finalize()
    return nc


def _body(nc, tc, g, use_bv, ln_affine, use_bias):
    ctx = contextlib.ExitStack()
    with ctx:
        # ---- SBUF pools (per-partition bytes in comments) ----
        wp = ctx.enter_context(tc.tile_pool(name="wp", bufs=24))      # 2KB*24 = 48KB
        h1p = ctx.enter_context(tc.tile_pool(name="h1p", bufs=1))    # 32KB
        xbp = ctx.enter_context(tc.tile_pool(name="xbp", bufs=1))    # 1KB*8 = 8KB
        qkp = ctx.enter_context(tc.tile_pool(name="qkp", bufs=1))    # 1KB*16 = 16KB
        vp = ctx.enter_context(tc.tile_pool(name="vp", bufs=1))      # ~2KB*4 = 8.2KB
        esp = ctx.enter_context(tc.tile_pool(name="esp", bufs=4))    # 1KB*4 = 4KB
        rp = ctx.enter_context(tc.tile_pool(name="rp", bufs=1))      # 2KB*8 = 16KB
        op = ctx.enter_context(tc.tile_pool(name="op", bufs=1))      # 1KB*8 = 8KB
        sqp = ctx.enter_context(tc.tile_pool(name="sqp", bufs=3))    # 2KB*3 = 6KB
        dp = ctx.enter_context(tc.tile_pool(name="dp", bufs=2))      # 2KB*2 = 4KB
        rowp = ctx.enter_context(tc.tile_pool(name="rowp", bufs=1))  # tiny
        cstp = ctx.enter_context(tc.tile_pool(name="cstp", bufs=2))  # tiny
        onep = ctx.enter_context(tc.tile_pool(name="onep", bufs=1))  # consts
        bcp = ctx.enter_context(tc.tile_pool(name="bcp", bufs=2))    # 2KB*2 pos stream

        # ---- PSUM: one rotating set of 8 banks ----
        psp = ctx.enter_context(tc.tile_pool(name="psp", bufs=8, space="PSUM"))

        def ps_tile(name):
            return psp.tile([P, N], F32, name=name, tag="mm")

        # ---- constants ----
        ones_col = onep.tile([P, 1], F32R, name="ones_col", tag="ones_col")
        nc.sync.dma_start(out=ones_col, in_=g["ones_col"][:, :])
        ones_row = onep.tile([1, P], F32R, name="ones_row", tag="ones_row")
        nc.sync.dma_start(out=ones_row, in_=g["ones_row"][:, :])
        ident = onep.tile([P, P], CDT, name="ident", tag="ident")
        nc.sync.dma_start(out=ident, in_=g["ident"][:, :])
        ssel = onep.tile([H, D], F32R, name="ssel", tag="ssel")
        nc.sync.dma_start(out=ssel, in_=g["ssel"][:, :])
        qm16 = onep.tile([H, N], F32, name="qm16", tag="qm16")
        nc.sync.dma_start(out=qm16, in_=g["qm16"][:, :])
        mneg = []
        for kc in range(HT):
            mt = onep.tile([P, N], CDT, name=f"mneg_{kc}", tag=f"mneg_{kc}")
            nc.sync.dma_start(out=mt, in_=g["maskneg"][kc])
            mneg.append(mt)
        eps30 = onep.tile([1, 1], F32, name="eps30", tag="eps30")
        nc.vector.memset(eps30, 1e-30)

        def bias_bundle(vec_ap, ncols, name):
            tl = cstp.tile([P, ncols], F32, name=name, tag="bias_bundle", bufs=6)
            nc.sync.dma_start(out=tl, in_=vec_ap.rearrange("(m p) -> p m", p=P))
            return tl

        # =============== embedding MLP ===============
        GK = [(0, 128), (128, 128), (256, GD - 256)]
        e_tiles = []
        for i, (k0, kn) in enumerate(GK):
            et = onep.tile([P, N], CDT, name=f"et_{i}", tag=f"emb_{i}")
            nc.sync.dma_start(out=et[:kn, :], in_=g["eT"][k0:k0 + kn, :])
            e_tiles.append((et, kn))

        mb1 = bias_bundle(g["mlp_b1"][:], MLP_H // P, "mb1") if use_bias else None
        h0 = h1p.tile([P, (MLP_H // P) * N], CDT, name="h0", tag="h1")
        for ph in range(2):
            w1t = []
            for i, (k0, kn) in enumerate(GK):
                w = wp.tile([P, 1024], CDT, name=f"mw1_{ph}_{i}", tag="w")
                nc.sync.dma_start(out=w[:kn, :],
                                  in_=g["mlp_w1"][k0:k0 + kn, ph * 1024:(ph + 1) * 1024])
                w1t.append((w, kn))
            for mm in range(8):
                m = ph * 8 + mm
                ps = ps_tile("mlp1_ps")
                for i, (_, kn) in enumerate(GK):
                    nc.tensor.matmul(ps, w1t[i][0][:kn, mm * P:(mm + 1) * P],
                                     e_tiles[i][0][:kn, :],
                                     start=(i == 0), stop=(i == len(GK) - 1))
                if use_bias:
                    nc.scalar.activation(h0[:, m * N:(m + 1) * N], ps, AF.Relu,
                                         bias=mb1[:, m:m + 1])
                else:
                    nc.scalar.activation(h0[:, m * N:(m + 1) * N], ps, AF.Relu)

        mb2 = bias_bundle(g["mlp_b2"][:], DT_TILES, "mb2") if use_bias else None
        MK = MLP_H // P  # 16
        pss = {m: ps_tile(f"mlp2_ps_{m}") for m in range(DT_TILES)}
        for kg in range(2):
            w2t = []
            for j in range(8):
                k = kg * 8 + j
                w = wp.tile([P, D], CDT, name=f"mw2_{k}", tag="w")
                nc.sync.dma_start(out=w, in_=g["mlp_w2"][k * P:(k + 1) * P, :])
                w2t.append(w)
            for j in range(8):
                k = kg * 8 + j
                for m in range(DT_TILES):
                    nc.tensor.matmul(pss[m], w2t[j][:, m * P:(m + 1) * P],
                                     h0[:, k * N:(k + 1) * N],
                                     start=(k == 0), stop=(k == MK - 1))
        x_bf = []
        for m in range(DT_TILES):
            pos_m = bcp.tile([P, N], F32, name=f"pos_{m}", tag="pos")
            nc.sync.dma_start(out=pos_m, in_=g["posT"][m * P:(m + 1) * P, :])
            r0 = rp.tile([P, N], F32R, name=f"r0_{m}", tag=f"r_{m}")
            if use_bias:
                nc.vector.scalar_tensor_tensor(r0, pss[m], mb2[:, m:m + 1], pos_m,
                                               op0=ALU.add, op1=ALU.add)
            else:
                nc.vector.tensor_add(r0, pss[m], pos_m)
            xb = xbp.tile([P, N], CDT, name=f"x0b_{m}", tag=f"x_{m}")
            nc.vector.tensor_copy(xb, r0)
            x_bf.append(xb)

        # =============== transformer blocks ===============
        for blk in range(N_BLOCKS):
            bq_b = bias_bundle(g["bq"][blk, :], DT_TILES, f"bq_{blk}") if use_bias else None
            bk_b = bias_bundle(g["bk"][blk, :], DT_TILES, f"bk_{blk}") if use_bias else None

            # ---- q/k projections, feature-major ----
            qT = [qkp.tile([P, N], CDT, name=f"q{blk}_{m}", tag=f"q_{m}") for m in range(DT_TILES)]
            kTt = [qkp.tile([P, N], CDT, name=f"k{blk}_{m}", tag=f"k_{m}") for m in range(DT_TILES)]
            for wname, bb, dst in (("wq", bq_b, qT), ("wk", bk_b, kTt)):
                wt = []
                for k in range(DT_TILES):
                    w = wp.tile([P, D], CDT, name=f"{wname}{blk}_{k}", tag="w")
                    nc.sync.dma_start(out=w, in_=g[wname][blk, k * P:(k + 1) * P, :])
                    wt.append(w)
                for m in range(DT_TILES):
                    ps = ps_tile(f"{wname}_ps")
                    for k in range(DT_TILES):
                        nc.tensor.matmul(ps, wt[k][:, m * P:(m + 1) * P], x_bf[k],
                                         start=(k == 0), stop=(k == DT_TILES - 1))
                    if use_bias:
                        nc.scalar.activation(dst[m], ps, AF.Relu, bias=bb[:, m:m + 1])
                    else:
                        nc.scalar.activation(dst[m], ps, AF.Relu)

            # ---- v projection, token-major, per-head layout with ones cols ----
            wvt = []
            for k in range(DT_TILES):
                w = wp.tile([P, D], CDT, name=f"wv{blk}_{k}", tag="w")
                nc.sync.dma_start(out=w, in_=g["wv"][blk, k * P:(k + 1) * P, :])
                wvt.append(w)
            if use_bv:
                bv_row = rowp.tile([1, D], F32, name=f"bvr_{blk}", tag="row_bv", bufs=1)
                nc.sync.dma_start(out=bv_row, in_=g["bv"][blk:blk + 1, :])
                bv_bc = bcp.tile([P, D], F32, name=f"bvb_{blk}", tag="bc_bv", bufs=2)
                nc.gpsimd.partition_broadcast(bv_bc, bv_row)
            vt = [vp.tile([P, VCOLS], CDT, name=f"v{blk}_{tt}", tag=f"v_{tt}")
                  for tt in range(BPC * HT)]
            for tt in range(BPC * HT):
                ones_ap = vt[tt].rearrange("p (h c) -> p h c", h=H)[:, :, DH:VH]
                nc.vector.memset(ones_ap, 1.0)
                for half in range(2):
                    ps = ps_tile("v_ps")
                    c0 = half * (D // 2)
                    for k in range(DT_TILES):
                        nc.tensor.matmul(ps, x_bf[k][:, tt * P:(tt + 1) * P],
                                         wvt[k][:, c0:c0 + D // 2],
                                         start=(k == 0), stop=(k == DT_TILES - 1))
                    dst = vt[tt].rearrange("p (h c) -> p h c", h=H)[
                        :, half * (H // 2):(half + 1) * (H // 2), 0:DH]
                    src = ps[:, :D // 2]
                    if use_bv:
                        tmp = sqp.tile([P, D // 2], F32, name="v_tmp", tag="sq")
                        nc.vector.tensor_add(tmp, src, bv_bc[:, c0:c0 + D // 2])
                        src = tmp
                    nc.scalar.activation(
                        dst, src.rearrange("p (h c) -> p h c", c=DH), AF.Relu)

            # ---- attention ----
            o_acc = [op.tile([P, N], CDT, name=f"o{blk}_{m}", tag=f"o_{m}")
                     for m in range(DT_TILES)]
            den = dp.tile([H, N], F32, name=f"den_{blk}", tag="den")

            def emit_scores(h):
                """scores for head h, both key chunks; mask pre-accumulated."""
                es = []
                ft, fo = h // 2, (h % 2) * DH
                for kc in range(HT):
                    ps = ps_tile("s_ps")
                    nc.tensor.matmul(ps, ident, mneg[kc],
                                     start=True, stop=False, skip_group_check=True)
                    for b in range(BPC):
                        nc.tensor.matmul(
                            ps[:, b * T:(b + 1) * T],
                            kTt[ft][fo:fo + DH, b * T + kc * P: b * T + (kc + 1) * P],
                            qT[ft][fo:fo + DH, b * T:(b + 1) * T],
                            start=False, stop=(b == BPC - 1), skip_group_check=True)
                    ex = esp.tile([P, N], CDT, name="expS", tag="es")
                    nc.scalar.activation(ex, ps, AF.Exp, scale=SCALE)
                    es.append(ex)
                return es

            def emit_out(h, es):
                ft, fo = h // 2, (h % 2) * DH
                ob = ps_tile("o_head_ps")
                for b in range(BPC):
                    for kc in range(HT):
                        nc.tensor.matmul(ob[:VH, b * T:(b + 1) * T],
                                         vt[b * HT + kc][:, h * VH:(h + 1) * VH],
                                         es[kc][:, b * T:(b + 1) * T],
                                         start=(kc == 0), stop=(kc == HT - 1))
                # o rows -> o_acc (bf16); denominator row -> partition-0 row
                # tile (engines cannot write non-32-aligned partition bases),
                # then a tiny gpsimd-queue DMA drops it into den[h].
                nc.vector.tensor_copy(o_acc[ft][fo:fo + DH, :], ob[0:DH, :])
                drow = rowp.tile([1, N], F32, name="drow", tag="drow", bufs=4)
                nc.scalar.activation(drow, ob[DH:VH, :], AF.Identity, bias=eps30)
                nc.gpsimd.dma_start(out=den[h:h + 1, :], in_=drow)

            prev = None
            for h in range(H):
                es_h = emit_scores(h)
                if prev is not None:
                    emit_out(*prev)
                prev = (h, es_h)
            emit_out(*prev)

            # ---- normalizer: qmask / den, broadcast to feature layout ----
            rden = dp.tile([H, N], F32, name=f"rden_{blk}", tag="rden")
            nc.vector.reciprocal_approx_fast(rden, den)
            rden_r = dp.tile([H, N], F32R, name=f"rdenr_{blk}", tag="rdenr")
            nc.vector.tensor_mul(rden_r, rden, qm16)
            r_new = []
            for ft in range(DT_TILES):
                nb = ps_tile("norm_ps")
                nc.tensor.matmul(nb, ssel[:, ft * P:(ft + 1) * P],
                                 rden_r, start=True, stop=True)
                nc.vector.tensor_mul(o_acc[ft], o_acc[ft], nb)
                r1 = rp.tile([P, N], F32R, name=f"r1_{blk}_{ft}", tag=f"r_{ft}")
                nc.vector.tensor_add(r1, o_acc[ft], x_bf[ft])
                r_new.append(r1)
            x_bf, _ = _layernorm(nc, g, blk, "ln1", r_new, ones_col, ones_row,
                                 xbp, sqp, rowp, cstp, psp, None, ln_affine)

            # ---- FFN up (4 column passes of 1024) ----
            fb1 = bias_bundle(g["ff_b1"][blk, :], FF_TILES, f"fb1_{blk}") if use_bias else None
            h1 = h1p.tile([P, FF_TILES * N], CDT, name=f"h1_{blk}", tag="h1")
            for ph in range(4):
                w1t = []
                for k in range(DT_TILES):
                    w = wp.tile([P, D], CDT, name=f"fw1_{blk}_{ph}_{k}", tag="w")
                    nc.sync.dma_start(
                        out=w, in_=g["ff_w1"][blk, k * P:(k + 1) * P,
                                              ph * 1024:(ph + 1) * 1024])
                    w1t.append(w)
                for mm in range(8):
                    m = ph * 8 + mm
                    ps = ps_tile("ff1_ps")
                    for k in range(DT_TILES):
                        nc.tensor.matmul(ps, w1t[k][:, mm * P:(mm + 1) * P], x_bf[k],
                                         start=(k == 0), stop=(k == DT_TILES - 1))
                    if use_bias:
                        nc.scalar.activation(h1[:, m * N:(m + 1) * N], ps, AF.Relu,
                                             bias=fb1[:, m:m + 1])
                    else:
                        nc.scalar.activation(h1[:, m * N:(m + 1) * N], ps, AF.Relu)

            # ---- FFN down: kg 0-2 j-major, kg 3 m-major for early drains ----
            fb2 = bias_bundle(g["ff_b2"][blk, :], DT_TILES, f"fb2_{blk}") if use_bias else None
            pss = {m: ps_tile(f"ff2_ps_{m}") for m in range(DT_TILES)}
            w2_last = None
            for kg in range(4):
                w2t = []
                for j in range(8):
                    k = kg * 8 + j
                    w = wp.tile([P, D], CDT, name=f"fw2_{blk}_{k}", tag="w")
                    nc.sync.dma_start(out=w,
                                      in_=g["ff_w2"][blk, k * P:(k + 1) * P, :])
                    w2t.append(w)
                if kg < 3:
                    for j in range(8):
                        k = kg * 8 + j
                        for m in range(DT_TILES):
                            nc.tensor.matmul(pss[m], w2t[j][:, m * P:(m + 1) * P],
                                             h1[:, k * N:(k + 1) * N],
                                             start=(k == 0), stop=False)
                else:
                    w2_last = w2t
            # last k-group m-major: each pss[m] chain closes early so its
            # drain/square/stat-matmuls overlap the remaining chains.  The
            # sums/sumsq PSUM tiles reuse the slots of pss[0]/pss[1], so they
            # are allocated (and their chains started) only after those two
            # have drained -- otherwise the PE FIFO deadlocks.
            r_new = []
            sq_tiles = []
            sums = sumsq = None
            for m in range(DT_TILES):
                for j in range(8):
                    k = 24 + j
                    nc.tensor.matmul(pss[m], w2_last[j][:, m * P:(m + 1) * P],
                                     h1[:, k * N:(k + 1) * N],
                                     start=False, stop=(j == 7))
                r2 = rp.tile([P, N], F32R, name=f"r2_{blk}_{m}", tag=f"r_{m}")
                if use_bias:
                    t = sqp.tile([P, N], F32, name="ff2t", tag="sq")
                    nc.vector.scalar_tensor_tensor(t, pss[m], fb2[:, m:m + 1],
                                                   x_bf[m], op0=ALU.add, op1=ALU.add)
                    nc.vector.tensor_copy(r2, t)
                else:
                    nc.vector.tensor_add(r2, pss[m], x_bf[m])
                sq = sqp.tile([P, N], F32R, name="ln2sq", tag="sq")
                nc.scalar.square(sq, r2)
                r_new.append(r2)
                sq_tiles.append(sq)
                if m == 1:
                    sums = psp.tile([P, N], F32, name=f"ln2s_{blk}", tag="mm")[0:1, :]
                    sumsq = psp.tile([P, N], F32, name=f"ln2q_{blk}", tag="mm")[0:1, :]
                    for mm_ in (0, 1):
                        nc.tensor.matmul(sums, ones_col, r_new[mm_],
                                         start=(mm_ == 0), stop=False)
                        nc.tensor.matmul(sumsq, ones_col, sq_tiles[mm_],
                                         start=(mm_ == 0), stop=False)
                elif m > 1:
                    nc.tensor.matmul(sums, ones_col, r2,
                                     start=False, stop=(m == DT_TILES - 1))
                    nc.tensor.matmul(sumsq, ones_col, sq,
                                     start=False, stop=(m == DT_TILES - 1))
            last = blk == N_BLOCKS - 1
            x_bf, _ = _layernorm(nc, g, blk, "ln2", r_new, ones_col, ones_row,
                                 xbp, sqp, rowp, cstp, psp,
                                 g["out"] if last else None, ln_affine,
                                 stats=(sums, sumsq))


def _layernorm(nc, g, blk, which, r_tiles, ones_col, ones_row,
               xbp, sqp, rowp, cstp, psp, out_dram, affine, stats=None):
    """LN over the partition (feature) axis.  r_tiles are updated IN PLACE to
    r - mean (the residual stream stays mean-shifted; LN is invariant to
    per-token shifts so downstream statistics are unaffected)."""
    nt = len(r_tiles)
    if affine:
        gb = cstp.tile([P, nt], F32, name=f"{which}g_{blk}", tag="bias_bundle", bufs=6)
        nc.sync.dma_start(out=gb, in_=g[f"{which}_g"][blk, :].rearrange("(m p) -> p m", p=P))
        bb = cstp.tile([P, nt], F32, name=f"{which}b_{blk}", tag="bias_bundle", bufs=6)
        nc.sync.dma_start(out=bb, in_=g[f"{which}_b"][blk, :].rearrange("(m p) -> p m", p=P))

    if stats is None:
        sums = psp.tile([P, N], F32, name=f"{which}s_{blk}", tag="mm")[0:1, :]
        sumsq = psp.tile([P, N], F32, name=f"{which}q_{blk}", tag="mm")[0:1, :]
        for m in range(nt):
            nc.tensor.matmul(sums, ones_col, r_tiles[m],
                             start=(m == 0), stop=(m == nt - 1))
        for m in range(nt):
            s = sqp.tile([P, N], F32R, name=f"{which}_sq", tag="sq")
            nc.scalar.square(s, r_tiles[m])
            nc.tensor.matmul(sumsq, ones_col, s,
                             start=(m == 0), stop=(m == nt - 1))
    else:
        sums, sumsq = stats

    mean = rowp.tile([1, N], F32R, name=f"{which}_mean", tag="row_a", bufs=2)
    nc.scalar.mul(mean, sums, 1.0 / D)
    # b_mean = ones_row.T @ mean  (PE broadcast, one bank)
    bm = psp.tile([P, N], F32, name=f"{which}_bm", tag="mm")
    nc.tensor.matmul(bm, ones_row, mean, start=True, stop=True)
    # var = sumsq/D - mean^2
    t = rowp.tile([1, N], F32, name=f"{which}_t", tag="row_b", bufs=2)
    nc.vector.scalar_tensor_tensor(t, mean, -1.0, mean, op0=ALU.mult, op1=ALU.mult)
    var = rowp.tile([1, N], F32, name=f"{which}_var", tag="row_c", bufs=2)
    nc.vector.scalar_tensor_tensor(var, sumsq, 1.0 / D, t, op0=ALU.mult, op1=ALU.add)
    # r -= b_mean (in place; residual stays shifted)
    for m in range(nt):
        nc.vector.tensor_sub(r_tiles[m], r_tiles[m], bm)
    # rstd = exp(-0.5*ln(var+eps)) -- same ACT table set as softmax exp
    eps_c = rowp.tile([1, 1], F32, name=f"{which}_eps", tag="row_eps", bufs=2)
    nc.vector.memset(eps_c, EPS)
    lnv = rowp.tile([1, N], F32, name=f"{which}_lnv", tag="row_d", bufs=2)
    nc.scalar.activation(lnv, var, AF.Ln, bias=eps_c)
    rstd = rowp.tile([1, N], F32R, name=f"{which}_rstd", tag="row_e", bufs=2)
    nc.scalar.activation(rstd, lnv, AF.Exp, scale=-0.5)
    br = psp.tile([P, N], F32, name=f"{which}_br", tag="mm")
    nc.tensor.matmul(br, ones_row, rstd, start=True, stop=True)

    xb_out = []
    for m in range(nt):
        if out_dram is not None:
            xo = sqp.tile([P, N], F32, name=f"{which}_xo", tag="sq")
            nc.vector.tensor_mul(xo, r_tiles[m], br)
            if affine:
                nc.vector.tensor_scalar(out=xo, in0=xo, scalar1=gb[:, m:m + 1],
                                        scalar2=bb[:, m:m + 1], op0=ALU.mult, op1=ALU.add)
            nc.sync.dma_start(out=out_dram[m * P:(m + 1) * P, :], in_=xo)
            xb_out.append(None)
        else:
            xb = xbp.tile([P, N], CDT, name=f"{which}_xb_{m}", tag=f"x_{m}")
            if affine:
                xf = sqp.tile([P, N], F32, name=f"{which}_xf", tag="sq")
                nc.vector.tensor_mul(xf, r_tiles[m], br)
                nc.vector.tensor_scalar(out=xb, in0=xf, scalar1=gb[:, m:m + 1],
                                        scalar2=bb[:, m:m + 1], op0=ALU.mult, op1=ALU.add)
            else:
                nc.vector.tensor_mul(xb, r_tiles[m], br)
            xb_out.append(xb)
    return xb_out, r_tiles


# ---------------------------------------------------------------------------
# host side
# ---------------------------------------------------------------------------

def _prepare_inputs(inputs):
    ipt = np.asarray(inputs["syb_ipt"]).astype(np.int64)
    emb = np.asarray(inputs["emb_table"], dtype=np.float32)
    smask = np.asarray(inputs["syb_mask"]).astype(np.int32)
    graph = np.asarray(inputs["syb_graph"]).astype(np.int32)

    gathered = emb[ipt]                                   # (B, T, GD)
    km = smask > 0
    M = (graph > 0) & km[:, None, :]                      # (B, Tq, Tk)
    MT = np.transpose(M, (0, 2, 1))                       # (B, Tk, Tq)
    qs = smask.astype(np.float32)

    posT = np.asarray(inputs["pos_table"], np.float32).T  # (D, T)
    posT2 = np.ascontiguousarray(np.tile(posT, (1, BPC)))

    # selection matrix: feature partition p of tile ft belongs to head 2ft+p//64
    ssel = np.zeros((H, D), np.float32)
    for ft in range(DT_TILES):
        for p in range(P):
            ssel[2 * ft + p // DH, ft * P + p] = 1.0

    def cvt(x):
        return np.ascontiguousarray(np.asarray(x, np.float32).astype(NPCDT))

    def f32(x):
        return np.ascontiguousarray(np.asarray(x, np.float32))

    common = {
        "posT": posT2,
        "ones_col": np.ones((P, 1), np.float32),
        "ones_row": np.ones((1, P), np.float32),
        "ident": np.eye(P, dtype=NPCDT),
        "ssel": ssel,
        "mlp_w1": cvt(inputs["mlp_w1"]), "mlp_b1": f32(inputs["mlp_b1"]),
        "mlp_w2": cvt(inputs["mlp_w2"]), "mlp_b2": f32(inputs["mlp_b2"]),
        "wq": cvt(inputs["wq"]), "wk": cvt(inputs["wk"]), "wv": cvt(inputs["wv"]),
        "bq": f32(inputs["bq"]), "bk": f32(inputs["bk"]), "bv": f32(inputs["bv"]),
        "ff_w1": cvt(inputs["ff_w1"]), "ff_b1": f32(inputs["ff_b1"]),
        "ff_w2": cvt(inputs["ff_w2"]), "ff_b2": f32(inputs["ff_b2"]),
        "ln1_g": f32(inputs["ln1_g"]), "ln1_b": f32(inputs["ln1_b"]),
        "ln2_g": f32(inputs["ln2_g"]), "ln2_b": f32(inputs["ln2_b"]),
    }
    use_bv = bool(np.any(np.asarray(inputs["bv"]) != 0))
    use_bias = bool(
        np.any(np.asarray(inputs["bq"]) != 0) or np.any(np.asarray(inputs["bk"]) != 0)
        or np.any(np.asarray(inputs["mlp_b1"]) != 0) or np.any(np.asarray(inputs["mlp_b2"]) != 0)
        or np.any(np.asarray(inputs["ff_b1"]) != 0) or np.any(np.asarray(inputs["ff_b2"]) != 0))
    ln_affine = bool(
        np.any(np.asarray(inputs["ln1_g"]) != 1) or np.any(np.asarray(inputs["ln1_b"]) != 0)
        or np.any(np.asarray(inputs["ln2_g"]) != 1) or np.any(np.asarray(inputs["ln2_b"]) != 0))

    in_maps = []
    for c in range(NCORES):
        b0 = c * BPC
        eT_c = np.ascontiguousarray(gathered[b0:b0 + BPC].reshape(N, GD).T).astype(NPCDT)
        # maskneg[kc][p, b*T + q] = NEG * (1 - M[b0+b, q, kc*128+p])
        mn = np.zeros((HT, P, N), np.float32)
        for kc in range(HT):
            for b in range(BPC):
                mn[kc, :, b * T:(b + 1) * T] = np.where(
                    MT[b0 + b, kc * P:(kc + 1) * P, :], 0.0, NEG)
        qm = np.broadcast_to(
            np.concatenate([qs[b0 + b] for b in range(BPC)])[None, :], (H, N))
        in_maps.append({
            "eT": eT_c,
            "maskneg": mn.astype(NPCDT),
            "qm16": np.ascontiguousarray(qm, dtype=np.float32),
            **common,
        })
    return in_maps, use_bv, ln_affine, use_bias


def _ensure_ntff_hook():
    """The agent image's antenv package lacks axon_hooks; synthesize it so
    run_bass_kernel_spmd(trace=True) can NTFF-profile through libaxon."""
    try:
        from antenv.axon_hooks import get_axon_ntff_profile_hook  # noqa: F401
        return
    except ImportError:
        pass
    try:
        import sys
        import types
        import antenv
        from trn_agent_boot.trn_boot import _ntff_profile_via_ctypes
        hook = _ntff_profile_via_ctypes("/opt/axon/libaxon_pjrt.so")
        mod = types.ModuleType("antenv.axon_hooks")
        mod._hook = hook
        mod.get_axon_ntff_profile_hook = lambda: mod._hook
        def _set(h):
            mod._hook = h
        mod.set_axon_ntff_profile_hook = _set
        sys.modules["antenv.axon_hooks"] = mod
        antenv.axon_hooks = mod
    except Exception as e:  # profiling is best-effort
        print(f"ntff hook injection failed: {e}")


def run(inputs, trace=False, tmpdir=None):
    in_maps, use_bv, ln_affine, use_bias = _prepare_inputs(inputs)
    nc = build_graph(use_bv, ln_affine, use_bias)
    if trace:
        _ensure_ntff_hook()
    res = run_bass_kernel_spmd(nc, in_maps, core_ids=list(range(NCORES)),
                               trace=trace, tmpdir=tmpdir)
    out = np.empty((B, T, D), np.float32)
    for c in range(NCORES):
        xT = np.asarray(res.results[c]["out"])            # (D, N)
        out[c * BPC:(c + 1) * BPC] = xT.T.reshape(BPC, T, D)
    return out, res


def kernel(**inputs):
    out, _ = run(inputs, trace=False)
    return out
